# revision 19
# baseline (speedup 1.0000x reference)
"""MoE gate kernel for Trainium2 (8 NeuronCores, SPMD).

Computes, for x [B=4, S=4096, D=2048] f32 and router weight [E=64, D=2048] f32:
    logits = x_flat @ weight.T          # [T=16384, 64]
    scores = softmax(logits)            # monotonic in logits
    topk_weight, topk_index = top_k(scores, 8), normalized over the top-8

Sharding: data-parallel over the flattened token dim (2048 tokens/core);
the tiny router weight is replicated (passed host-pre-transposed as [D, E]).

Per-core pipeline (fp32r mode):
  - DMA x tiles [128, 2048] f32 (natural layout, 8KB/partition descriptors,
    full HBM bandwidth); the contraction dim is chunked mod-16
    (chunk c = {d : d = 16*a + c}) so the router weight DMA needs only one
    4KB descriptor per partition
  - PE transposes 128x128 blocks (fp32 transpose-mode, bit-exact; the
    stationary reads stride-16 SBUF columns at no extra cost) -> f32 PSUM
  - PSUM -> SBUF copies (DVE/ACT halves) round to fp32r (tf32), enabling
    the full-rate (1 cyc/row) fp32r matmul path; logits err ~2e-4
  - transposes/copies are software-pipelined SKEW chunks ahead of the
    matmuls ACROSS group boundaries (no pipeline bubble per group)
  - fp32r matmul: logitsT[64, 512] accumulated in f32 PSUM over 16 chunks
  - PE-transpose logitsT back to [128 tokens, 64]
  - DVE max/max_index per token tile: top-8 values (descending) + indices
  - group-batched softmax over the top-8 (full-softmax denominator
    cancels; exp without max-shift is safe: |logit| < ~4)
  - outputs accumulated in SBUF, one DMA per tensor per group (SP engine)

The walrus LDWEIGHTS optimizer is enabled (see patch below) so stationary
weight loads overlap matmul streaming on the PE.
"""

import numpy as np

import concourse.bass as bass
import concourse.mybir as mybir
from concourse import bacc
from concourse.tile import TileContext
from concourse.bass_utils import run_bass_kernel_spmd
from concourse.masks import make_identity

# Enable the walrus LDWEIGHTS optimizer so stationary loads overlap with
# matmul streaming (with the default --enable-ldw-opt=false every matmul
# pays a serial ~NumWeights/1.2GHz weight-load, ~40% of PE time here).
import concourse.bass_utils as _bu

if not getattr(_bu, "_ldw_opt_patched", False):
    _orig_run_command = _bu.run_command

    def _run_command_ldw(argv, **kw):
        argv = [
            "--enable-ldw-opt=true" if a == "--enable-ldw-opt=false" else a
            for a in argv
        ]
        return _orig_run_command(argv, **kw)

    _bu.run_command = _run_command_ldw
    _bu._ldw_opt_patched = True

N_CORES = 8
T_FULL = 16384          # total tokens (4 * 4096)
T_LOC = T_FULL // N_CORES  # 2048 tokens per core
D = 2048
E = 64
TOPK = 8
GROUP_T = 512                    # tokens per matmul group (PSUM bank width)
N_GROUPS = T_LOC // GROUP_T      # 4
TPG = GROUP_T // 128             # token tiles per group: 4
N_CHUNKS = D // 128              # contraction chunks: 16

_F32 = mybir.dt.float32
_F32R = mybir.dt.float32r
_U32 = mybir.dt.uint32

MODE = "fp32r"   # "fp32r" | "split3"  (speed vs logits accuracy)
SKEW = 4         # transposed slabs kept in flight ahead of the matmul


def _build(trace_label=None):
    nc = bacc.Bacc(num_devices=N_CORES)

    # In fp32r mode x is declared float32r end-to-end: the DMA moves the
    # same f32 bytes, and the PE transpose runs in fp32r mode (1.5 cyc/row
    # instead of 2), which is where the tf32 rounding happens.
    x_dt = _F32R if MODE == "fp32r" else _F32
    x = nc.declare_dram_parameter("x", [T_LOC, D], x_dt, isOutput=False)
    wT = nc.declare_dram_parameter("wT", [D, E], _F32, isOutput=False)
    topw = nc.declare_dram_parameter("topw", [T_LOC, TOPK], _F32, isOutput=True)
    topi = nc.declare_dram_parameter("topi", [T_LOC, TOPK], _U32, isOutput=True)

    with TileContext(nc) as tc:
        with (
            tc.tile_pool(name="const", bufs=1) as cpool,
            tc.tile_pool(name="xin", bufs=8) as xpool,
            tc.tile_pool(name="xt", bufs=SKEW + 2) as xtpool,
            # (xt bufs track SKEW; ps_tp bufs = SKEW + 1)
            tc.tile_pool(name="small", bufs=3) as spool,
            tc.tile_pool(name="tiny", bufs=3) as tpool,
            tc.tile_pool(name="ps_tp", bufs=5, space="PSUM") as ps_tp,
            tc.tile_pool(name="ps_mm", bufs=2, space="PSUM") as ps_mm,
            tc.tile_pool(name="ps_lt", bufs=1, space="PSUM") as ps_lt,
        ):
            # x tile DMAs for the first two groups go out before anything
            # else so the PE can start transposing ASAP
            tiles = {}

            def load_group(g):
                if g in tiles or g >= N_GROUPS:
                    return
                ts = []
                for t in range(TPG):
                    xt = xpool.tile([128, D], x_dt, tag="x")
                    row0 = (g * TPG + t) * 128
                    nc.sync.dma_start(out=xt[:], in_=x[row0:row0 + 128, :])
                    ts.append(xt)
                tiles[g] = ts

            load_group(0)
            load_group(1)

            # router weight, chunked mod-16: wt_sb[p, c, e] = wT[16p + c, e]
            # -> one contiguous 4KB descriptor per partition
            wt_sb = cpool.tile([128, N_CHUNKS, E], _F32)
            nc.scalar.dma_start(
                out=wt_sb[:], in_=wT.rearrange("(p c) e -> p c e", p=128)
            )
            ident = cpool.tile([128, 128], _F32)
            make_identity(nc, ident[:])
            if MODE == "fp32r":
                ident_r = cpool.tile([128, 128], _F32R)
                nc.vector.tensor_copy(ident_r[:], ident[:])
            else:
                ident_r = ident

            # round the replicated router weight to fp32r once
            wt_hi = cpool.tile([128, N_CHUNKS, E], _F32R)
            nc.vector.tensor_copy(wt_hi[:], wt_sb[:])
            if MODE == "split3":
                wt_lo = cpool.tile([128, N_CHUNKS, E], _F32R)
                nc.vector.tensor_tensor(
                    out=wt_lo[:], in0=wt_sb[:], in1=wt_hi[:].bitcast(_F32),
                    op=mybir.AluOpType.subtract,
                )

            H = GROUP_T // 2

            # transpose chunk c of group g's 4 token tiles into one
            # [128, 512] f32 PSUM slab, then round into fp32r SBUF slab(s).
            # chunk c covers d = {16*a + c}: stride-16 column reads (free on
            # SBUF), matching wt_sb's layout.
            def make_xt(g, c):
                pt = ps_tp.tile([128, GROUP_T], x_dt, tag="tp")
                for t in range(TPG):
                    xv = tiles[g][t][:].rearrange("p (a b) -> p b a", b=N_CHUNKS)
                    nc.tensor.transpose(
                        pt[:, t * 128:(t + 1) * 128],
                        xv[:, c, :],
                        ident_r[:],
                    )
                hi = xtpool.tile([128, GROUP_T], _F32R, tag="xhi")
                if MODE != "split3":
                    nc.vector.tensor_copy(hi[:, 0:H], pt[:, 0:H])
                    nc.scalar.copy(out=hi[:, H:], in_=pt[:, H:])
                    return (hi,)
                nc.scalar.copy(out=hi[:], in_=pt[:])
                lo = xtpool.tile([128, GROUP_T], _F32R, tag="xlo")
                nc.vector.tensor_tensor(
                    out=lo[:], in0=pt[:], in1=hi[:].bitcast(_F32),
                    op=mybir.AluOpType.subtract,
                )
                return (hi, lo)

            n_mm = 3 if MODE == "split3" else 1
            NTOT = N_GROUPS * N_CHUNKS
            slabs = {}
            for f in range(SKEW):
                slabs[f] = make_xt(f // N_CHUNKS, f % N_CHUNKS)

            for g in range(N_GROUPS):
                logits_ps = ps_mm.tile([E, GROUP_T], _F32, tag="lg")
                for c in range(N_CHUNKS):
                    flat = g * N_CHUNKS + c
                    ahead = flat + SKEW
                    if ahead < NTOT:
                        g2, c2 = divmod(ahead, N_CHUNKS)
                        if c2 == 0:
                            load_group(g2 + 1)
                        slabs[ahead] = make_xt(g2, c2)
                    sl = slabs.pop(flat)
                    if MODE == "split3":
                        hi, lo = sl
                        parts = [
                            (wt_hi[:, c, :], hi[:]),
                            (wt_hi[:, c, :], lo[:]),
                            (wt_lo[:, c, :], hi[:]),
                        ]
                    else:
                        parts = [(wt_hi[:, c, :], sl[0][:])]
                    for j, (lhs, rhs) in enumerate(parts):
                        mm_i = c * n_mm + j
                        nc.tensor.matmul(
                            logits_ps[:], lhs, rhs,
                            start=(mm_i == 0),
                            stop=(mm_i == N_CHUNKS * n_mm - 1),
                        )

                # epilogue: transpose logitsT back to [tokens, E] in one PSUM
                # tile, then per-tile top-8 and one group-batched softmax
                lg_sb = spool.tile([E, GROUP_T], _F32, tag="lgsb")
                nc.scalar.copy(out=lg_sb[:], in_=logits_ps[:])
                lt_ps = ps_lt.tile([128, TPG, E], _F32, tag="lt")
                for t in range(TPG):
                    nc.tensor.transpose(
                        lt_ps[:, t, :],
                        lg_sb[:, t * 128:(t + 1) * 128],
                        ident[0:E, 0:E],
                    )
                lg_t = spool.tile([128, TPG, E], _F32, tag="lgt")
                nc.vector.tensor_copy(lg_t[:], lt_ps[:])

                m8 = tpool.tile([128, TPG, TOPK], _F32, tag="m8")
                i8 = tpool.tile([128, TPG, TOPK], _U32, tag="i8")
                for t in range(TPG):
                    nc.vector.max(out=m8[:, t, :], in_=lg_t[:, t, :])
                    nc.vector.max_index(
                        out=i8[:, t, :], in_max=m8[:, t, :], in_values=lg_t[:, t, :]
                    )

                # softmax over the top-8, batched across the 4 token tiles
                e8 = tpool.tile([128, TPG, TOPK], _F32, tag="e8")
                nc.scalar.activation(
                    e8[:], m8[:],
                    mybir.ActivationFunctionType.Exp, scale=1.0,
                )
                s1 = tpool.tile([128, TPG, 1], _F32, tag="s1")
                nc.vector.reduce_sum(s1[:], e8[:], axis=mybir.AxisListType.X)
                rc = tpool.tile([128, TPG, 1], _F32, tag="rc")
                nc.vector.reciprocal(rc[:], s1[:])
                w8 = tpool.tile([128, TPG, TOPK], _F32, tag="w8")
                nc.vector.tensor_tensor(
                    out=w8[:], in0=e8[:],
                    in1=rc[:].broadcast_to([128, TPG, TOPK]),
                    op=mybir.AluOpType.mult,
                )

                nc.sync.dma_start(
                    out=topw.rearrange("(n p) k -> p n k", p=128)[:, g * TPG:(g + 1) * TPG, :],
                    in_=w8[:],
                )
                nc.sync.dma_start(
                    out=topi.rearrange("(n p) k -> p n k", p=128)[:, g * TPG:(g + 1) * TPG, :],
                    in_=i8[:],
                )

    nc.compile()
    return nc


_NC_CACHE = {}


def _get_nc():
    if "nc" not in _NC_CACHE:
        _NC_CACHE["nc"] = _build()
    return _NC_CACHE["nc"]


def kernel(x: np.ndarray, weight: np.ndarray, _trace=False, _trace_kwargs=None):
    assert x.shape == (4, 4096, D) and weight.shape == (E, D)
    xf = np.ascontiguousarray(x.reshape(T_FULL, D), dtype=np.float32)
    wTv = np.ascontiguousarray(weight.astype(np.float32, copy=False).T)

    nc = _get_nc()
    in_maps = [
        {"x": xf[k * T_LOC:(k + 1) * T_LOC], "wT": wTv}
        for k in range(N_CORES)
    ]
    res = run_bass_kernel_spmd(
        nc, in_maps, list(range(N_CORES)),
        trace=_trace, **(_trace_kwargs or {}),
    )
    topw = np.concatenate([res.results[k]["topw"] for k in range(N_CORES)], axis=0)
    topi = np.concatenate(
        [res.results[k]["topi"].astype(np.int32) for k in range(N_CORES)], axis=0
    )
    if _trace:
        kernel.last_exec_time_ns = res.exec_time_ns
        kernel.last_results = res
    return topw, topi


# revision 20
# speedup vs baseline: 1.0022x; 1.0022x over previous
"""MoE gate kernel for Trainium2 (8 NeuronCores, SPMD).

Computes, for x [B=4, S=4096, D=2048] f32 and router weight [E=64, D=2048] f32:
    logits = x_flat @ weight.T          # [T=16384, 64]
    scores = softmax(logits)            # monotonic in logits
    topk_weight, topk_index = top_k(scores, 8), normalized over the top-8

Sharding: data-parallel over the flattened token dim (2048 tokens/core);
the tiny router weight is replicated (passed host-pre-transposed as [D, E]).

Per-core pipeline (fp32r mode):
  - DMA x tiles [128, 2048] f32 (natural layout, 8KB/partition descriptors,
    full HBM bandwidth); the contraction dim is chunked mod-16
    (chunk c = {d : d = 16*a + c}) so the router weight DMA needs only one
    4KB descriptor per partition
  - PE transposes 128x128 blocks (fp32 transpose-mode, bit-exact; the
    stationary reads stride-16 SBUF columns at no extra cost) -> f32 PSUM
  - PSUM -> SBUF copies (DVE/ACT halves) round to fp32r (tf32), enabling
    the full-rate (1 cyc/row) fp32r matmul path; logits err ~2e-4
  - transposes/copies are software-pipelined SKEW chunks ahead of the
    matmuls ACROSS group boundaries (no pipeline bubble per group)
  - fp32r matmul: logitsT[64, 512] accumulated in f32 PSUM over 16 chunks
  - PE-transpose logitsT back to [128 tokens, 64]
  - DVE max/max_index per token tile: top-8 values (descending) + indices
  - group-batched softmax over the top-8 (full-softmax denominator
    cancels; exp without max-shift is safe: |logit| < ~4)
  - outputs accumulated in SBUF, one DMA per tensor per group (SP engine)

The walrus LDWEIGHTS optimizer is enabled (see patch below) so stationary
weight loads overlap matmul streaming on the PE.
"""

import numpy as np

import concourse.bass as bass
import concourse.mybir as mybir
from concourse import bacc
from concourse.tile import TileContext
from concourse.bass_utils import run_bass_kernel_spmd
from concourse.masks import make_identity

# Enable the walrus LDWEIGHTS optimizer so stationary loads overlap with
# matmul streaming (with the default --enable-ldw-opt=false every matmul
# pays a serial ~NumWeights/1.2GHz weight-load, ~40% of PE time here).
import concourse.bass_utils as _bu

if not getattr(_bu, "_ldw_opt_patched", False):
    _orig_run_command = _bu.run_command

    def _run_command_ldw(argv, **kw):
        argv = [
            "--enable-ldw-opt=true" if a == "--enable-ldw-opt=false" else a
            for a in argv
        ]
        return _orig_run_command(argv, **kw)

    _bu.run_command = _run_command_ldw
    _bu._ldw_opt_patched = True

N_CORES = 8
T_FULL = 16384          # total tokens (4 * 4096)
T_LOC = T_FULL // N_CORES  # 2048 tokens per core
D = 2048
E = 64
TOPK = 8
GROUP_T = 512                    # tokens per matmul group (PSUM bank width)
N_GROUPS = T_LOC // GROUP_T      # 4
TPG = GROUP_T // 128             # token tiles per group: 4
N_CHUNKS = D // 128              # contraction chunks: 16

_F32 = mybir.dt.float32
_F32R = mybir.dt.float32r
_U32 = mybir.dt.uint32

MODE = "fp32r"   # "fp32r" | "split3"  (speed vs logits accuracy)
SKEW = 3         # transposed slabs kept in flight ahead of the matmul


def _build(trace_label=None):
    nc = bacc.Bacc(num_devices=N_CORES)

    # In fp32r mode x is declared float32r end-to-end: the DMA moves the
    # same f32 bytes, and the PE transpose runs in fp32r mode (1.5 cyc/row
    # instead of 2), which is where the tf32 rounding happens.
    x_dt = _F32R if MODE == "fp32r" else _F32
    x = nc.declare_dram_parameter("x", [T_LOC, D], x_dt, isOutput=False)
    wT = nc.declare_dram_parameter("wT", [D, E], _F32, isOutput=False)
    topw = nc.declare_dram_parameter("topw", [T_LOC, TOPK], _F32, isOutput=True)
    topi = nc.declare_dram_parameter("topi", [T_LOC, TOPK], _U32, isOutput=True)

    with TileContext(nc) as tc:
        with (
            tc.tile_pool(name="const", bufs=1) as cpool,
            tc.tile_pool(name="xin", bufs=8) as xpool,
            tc.tile_pool(name="xt", bufs=SKEW + 2) as xtpool,
            # (xt bufs track SKEW; ps_tp bufs = SKEW + 1)
            tc.tile_pool(name="small", bufs=3) as spool,
            tc.tile_pool(name="tiny", bufs=3) as tpool,
            tc.tile_pool(name="ps_tp", bufs=4, space="PSUM") as ps_tp,
            tc.tile_pool(name="ps_mm", bufs=2, space="PSUM") as ps_mm,
            tc.tile_pool(name="ps_lt", bufs=2, space="PSUM") as ps_lt,
        ):
            # x tile DMAs for the first two groups go out before anything
            # else so the PE can start transposing ASAP
            tiles = {}

            def load_group(g):
                if g in tiles or g >= N_GROUPS:
                    return
                ts = []
                for t in range(TPG):
                    xt = xpool.tile([128, D], x_dt, tag="x")
                    row0 = (g * TPG + t) * 128
                    nc.sync.dma_start(out=xt[:], in_=x[row0:row0 + 128, :])
                    ts.append(xt)
                tiles[g] = ts

            load_group(0)
            load_group(1)

            # router weight, chunked mod-16: wt_sb[p, c, e] = wT[16p + c, e]
            # -> one contiguous 4KB descriptor per partition
            wt_sb = cpool.tile([128, N_CHUNKS, E], _F32)
            nc.scalar.dma_start(
                out=wt_sb[:], in_=wT.rearrange("(p c) e -> p c e", p=128)
            )
            ident = cpool.tile([128, 128], _F32)
            make_identity(nc, ident[:])
            if MODE == "fp32r":
                ident_r = cpool.tile([128, 128], _F32R)
                nc.vector.tensor_copy(ident_r[:], ident[:])
            else:
                ident_r = ident

            # round the replicated router weight to fp32r once
            wt_hi = cpool.tile([128, N_CHUNKS, E], _F32R)
            nc.vector.tensor_copy(wt_hi[:], wt_sb[:])
            if MODE == "split3":
                wt_lo = cpool.tile([128, N_CHUNKS, E], _F32R)
                nc.vector.tensor_tensor(
                    out=wt_lo[:], in0=wt_sb[:], in1=wt_hi[:].bitcast(_F32),
                    op=mybir.AluOpType.subtract,
                )

            H = GROUP_T // 2

            # transpose chunk c of group g's 4 token tiles into one
            # [128, 512] f32 PSUM slab, then round into fp32r SBUF slab(s).
            # chunk c covers d = {16*a + c}: stride-16 column reads (free on
            # SBUF), matching wt_sb's layout.
            def make_xt(g, c):
                pt = ps_tp.tile([128, GROUP_T], x_dt, tag="tp")
                for t in range(TPG):
                    xv = tiles[g][t][:].rearrange("p (a b) -> p b a", b=N_CHUNKS)
                    nc.tensor.transpose(
                        pt[:, t * 128:(t + 1) * 128],
                        xv[:, c, :],
                        ident_r[:],
                    )
                hi = xtpool.tile([128, GROUP_T], _F32R, tag="xhi")
                if MODE != "split3":
                    nc.vector.tensor_copy(hi[:, 0:H], pt[:, 0:H])
                    nc.scalar.copy(out=hi[:, H:], in_=pt[:, H:])
                    return (hi,)
                nc.scalar.copy(out=hi[:], in_=pt[:])
                lo = xtpool.tile([128, GROUP_T], _F32R, tag="xlo")
                nc.vector.tensor_tensor(
                    out=lo[:], in0=pt[:], in1=hi[:].bitcast(_F32),
                    op=mybir.AluOpType.subtract,
                )
                return (hi, lo)

            n_mm = 3 if MODE == "split3" else 1
            NTOT = N_GROUPS * N_CHUNKS
            slabs = {}
            for f in range(SKEW):
                slabs[f] = make_xt(f // N_CHUNKS, f % N_CHUNKS)

            for g in range(N_GROUPS):
                logits_ps = ps_mm.tile([E, GROUP_T], _F32, tag="lg")
                for c in range(N_CHUNKS):
                    flat = g * N_CHUNKS + c
                    ahead = flat + SKEW
                    if ahead < NTOT:
                        g2, c2 = divmod(ahead, N_CHUNKS)
                        if c2 == 0:
                            load_group(g2 + 1)
                        slabs[ahead] = make_xt(g2, c2)
                    sl = slabs.pop(flat)
                    if MODE == "split3":
                        hi, lo = sl
                        parts = [
                            (wt_hi[:, c, :], hi[:]),
                            (wt_hi[:, c, :], lo[:]),
                            (wt_lo[:, c, :], hi[:]),
                        ]
                    else:
                        parts = [(wt_hi[:, c, :], sl[0][:])]
                    for j, (lhs, rhs) in enumerate(parts):
                        mm_i = c * n_mm + j
                        nc.tensor.matmul(
                            logits_ps[:], lhs, rhs,
                            start=(mm_i == 0),
                            stop=(mm_i == N_CHUNKS * n_mm - 1),
                        )

                # epilogue: transpose logitsT back to [tokens, E] in one PSUM
                # tile, then per-tile top-8 and one group-batched softmax
                lg_sb = spool.tile([E, GROUP_T], _F32, tag="lgsb")
                nc.scalar.copy(out=lg_sb[:], in_=logits_ps[:])
                lt_ps = ps_lt.tile([128, TPG, E], _F32, tag="lt")
                for t in range(TPG):
                    nc.tensor.transpose(
                        lt_ps[:, t, :],
                        lg_sb[:, t * 128:(t + 1) * 128],
                        ident[0:E, 0:E],
                    )
                lg_t = spool.tile([128, TPG, E], _F32, tag="lgt")
                nc.vector.tensor_copy(lg_t[:], lt_ps[:])

                m8 = tpool.tile([128, TPG, TOPK], _F32, tag="m8")
                i8 = tpool.tile([128, TPG, TOPK], _U32, tag="i8")
                for t in range(TPG):
                    nc.vector.max(out=m8[:, t, :], in_=lg_t[:, t, :])
                    nc.vector.max_index(
                        out=i8[:, t, :], in_max=m8[:, t, :], in_values=lg_t[:, t, :]
                    )

                # softmax over the top-8, batched across the 4 token tiles
                e8 = tpool.tile([128, TPG, TOPK], _F32, tag="e8")
                nc.scalar.activation(
                    e8[:], m8[:],
                    mybir.ActivationFunctionType.Exp, scale=1.0,
                )
                s1 = tpool.tile([128, TPG, 1], _F32, tag="s1")
                nc.vector.reduce_sum(s1[:], e8[:], axis=mybir.AxisListType.X)
                rc = tpool.tile([128, TPG, 1], _F32, tag="rc")
                nc.vector.reciprocal(rc[:], s1[:])
                w8 = tpool.tile([128, TPG, TOPK], _F32, tag="w8")
                nc.vector.tensor_tensor(
                    out=w8[:], in0=e8[:],
                    in1=rc[:].broadcast_to([128, TPG, TOPK]),
                    op=mybir.AluOpType.mult,
                )

                nc.sync.dma_start(
                    out=topw.rearrange("(n p) k -> p n k", p=128)[:, g * TPG:(g + 1) * TPG, :],
                    in_=w8[:],
                )
                nc.sync.dma_start(
                    out=topi.rearrange("(n p) k -> p n k", p=128)[:, g * TPG:(g + 1) * TPG, :],
                    in_=i8[:],
                )

    nc.compile()
    return nc


_NC_CACHE = {}


def _get_nc():
    if "nc" not in _NC_CACHE:
        _NC_CACHE["nc"] = _build()
    return _NC_CACHE["nc"]


def kernel(x: np.ndarray, weight: np.ndarray, _trace=False, _trace_kwargs=None):
    assert x.shape == (4, 4096, D) and weight.shape == (E, D)
    xf = np.ascontiguousarray(x.reshape(T_FULL, D), dtype=np.float32)
    wTv = np.ascontiguousarray(weight.astype(np.float32, copy=False).T)

    nc = _get_nc()
    in_maps = [
        {"x": xf[k * T_LOC:(k + 1) * T_LOC], "wT": wTv}
        for k in range(N_CORES)
    ]
    res = run_bass_kernel_spmd(
        nc, in_maps, list(range(N_CORES)),
        trace=_trace, **(_trace_kwargs or {}),
    )
    topw = np.concatenate([res.results[k]["topw"] for k in range(N_CORES)], axis=0)
    topi = np.concatenate(
        [res.results[k]["topi"].astype(np.int32) for k in range(N_CORES)], axis=0
    )
    if _trace:
        kernel.last_exec_time_ns = res.exec_time_ns
        kernel.last_results = res
    return topw, topi


# revision 21
# speedup vs baseline: 1.0666x; 1.0643x over previous
"""MoE gate kernel for Trainium2 (8 NeuronCores, SPMD).

Computes, for x [B=4, S=4096, D=2048] f32 and router weight [E=64, D=2048] f32:
    logits = x_flat @ weight.T          # [T=16384, 64]
    scores = softmax(logits)            # monotonic in logits
    topk_weight, topk_index = top_k(scores, 8), normalized over the top-8

Sharding: data-parallel over the flattened token dim (2048 tokens/core);
the tiny router weight is replicated (passed host-pre-transposed as [D, E]).

Per-core pipeline (fp32r mode):
  - DMA x tiles [128, 2048] f32 (natural layout, 8KB/partition descriptors,
    full HBM bandwidth); the contraction dim is chunked mod-16
    (chunk c = {d : d = 16*a + c}) so the router weight DMA needs only one
    4KB descriptor per partition
  - PE transposes 128x128 blocks (fp32 transpose-mode, bit-exact; the
    stationary reads stride-16 SBUF columns at no extra cost) -> f32 PSUM
  - PSUM -> SBUF copies (DVE/ACT halves) round to fp32r (tf32), enabling
    the full-rate (1 cyc/row) fp32r matmul path; logits err ~2e-4
  - transposes/copies are software-pipelined SKEW chunks ahead of the
    matmuls ACROSS group boundaries (no pipeline bubble per group)
  - fp32r matmul: logitsT[64, 512] accumulated in f32 PSUM over 16 chunks
  - PE-transpose logitsT back to [128 tokens, 64]
  - DVE max/max_index per token tile: top-8 values (descending) + indices
  - group-batched softmax over the top-8 (full-softmax denominator
    cancels; exp without max-shift is safe: |logit| < ~4)
  - outputs accumulated in SBUF, one DMA per tensor per group (SP engine)

The walrus LDWEIGHTS optimizer is enabled (see patch below) so stationary
weight loads overlap matmul streaming on the PE.
"""

import numpy as np

import concourse.bass as bass
import concourse.mybir as mybir
from concourse import bacc
from concourse.tile import TileContext
from concourse.bass_utils import run_bass_kernel_spmd
from concourse.masks import make_identity

# Enable the walrus LDWEIGHTS optimizer so stationary loads overlap with
# matmul streaming (with the default --enable-ldw-opt=false every matmul
# pays a serial ~NumWeights/1.2GHz weight-load, ~40% of PE time here).
import concourse.bass_utils as _bu

if not getattr(_bu, "_ldw_opt_patched", False):
    _orig_run_command = _bu.run_command

    def _run_command_ldw(argv, **kw):
        argv = [
            "--enable-ldw-opt=true" if a == "--enable-ldw-opt=false" else a
            for a in argv
        ]
        return _orig_run_command(argv, **kw)

    _bu.run_command = _run_command_ldw
    _bu._ldw_opt_patched = True

N_CORES = 8
T_FULL = 16384          # total tokens (4 * 4096)
T_LOC = T_FULL // N_CORES  # 2048 tokens per core
D = 2048
E = 64
TOPK = 8
GROUP_T = 512                    # tokens per matmul group (PSUM bank width)
N_GROUPS = T_LOC // GROUP_T      # 4
TPG = GROUP_T // 128             # token tiles per group: 4
N_CHUNKS = D // 128              # contraction chunks: 16

_F32 = mybir.dt.float32
_F32R = mybir.dt.float32r
_U32 = mybir.dt.uint32

MODE = "fp32r"   # "fp32r" | "split3"  (speed vs logits accuracy)
SKEW = 3         # transposed slabs kept in flight ahead of the matmul


def _build(trace_label=None):
    nc = bacc.Bacc(num_devices=N_CORES)

    x_dt = _F32
    x = nc.declare_dram_parameter("x", [T_LOC, D], x_dt, isOutput=False)
    wT = nc.declare_dram_parameter("wT", [D, E], _F32, isOutput=False)
    topw = nc.declare_dram_parameter("topw", [T_LOC, TOPK], _F32, isOutput=True)
    topi = nc.declare_dram_parameter("topi", [T_LOC, TOPK], _U32, isOutput=True)

    with TileContext(nc) as tc:
        with (
            tc.tile_pool(name="const", bufs=1) as cpool,
            tc.tile_pool(name="xin", bufs=8) as xpool,
            tc.tile_pool(name="xt", bufs=SKEW + 2) as xtpool,
            # (xt bufs track SKEW; ps_tp bufs = SKEW + 1)
            tc.tile_pool(name="small", bufs=3) as spool,
            tc.tile_pool(name="tiny", bufs=3) as tpool,
            tc.tile_pool(name="ps_tp", bufs=4, space="PSUM") as ps_tp,
            tc.tile_pool(name="ps_mm", bufs=2, space="PSUM") as ps_mm,
            tc.tile_pool(name="ps_lt", bufs=2, space="PSUM") as ps_lt,
        ):
            # x tile DMAs for the first two groups go out before anything
            # else so the PE can start transposing ASAP
            tiles = {}

            def load_group(g):
                if g in tiles or g >= N_GROUPS:
                    return
                ts = []
                for t in range(TPG):
                    xt = xpool.tile([128, D], x_dt, tag="x")
                    row0 = (g * TPG + t) * 128
                    nc.sync.dma_start(out=xt[:], in_=x[row0:row0 + 128, :])
                    ts.append(xt)
                tiles[g] = ts

            load_group(0)
            load_group(1)

            # router weight, chunked mod-16: wt_sb[p, c, e] = wT[16p + c, e]
            # -> one contiguous 4KB descriptor per partition
            wt_sb = cpool.tile([128, N_CHUNKS, E], _F32)
            nc.scalar.dma_start(
                out=wt_sb[:], in_=wT.rearrange("(p c) e -> p c e", p=128)
            )
            ident = cpool.tile([128, 128], _F32)
            make_identity(nc, ident[:])
            ident_r = ident

            # round the replicated router weight to fp32r once
            wt_hi = cpool.tile([128, N_CHUNKS, E], _F32R)
            nc.vector.tensor_copy(wt_hi[:], wt_sb[:])
            if MODE == "split3":
                wt_lo = cpool.tile([128, N_CHUNKS, E], _F32R)
                nc.vector.tensor_tensor(
                    out=wt_lo[:], in0=wt_sb[:], in1=wt_hi[:].bitcast(_F32),
                    op=mybir.AluOpType.subtract,
                )

            H = GROUP_T // 2

            # transpose chunk c of group g's 4 token tiles into one
            # [128, 512] f32 PSUM slab, then round into fp32r SBUF slab(s).
            # chunk c covers d = {16*a + c}: stride-16 column reads (free on
            # SBUF), matching wt_sb's layout.
            def make_xt(g, c):
                pt = ps_tp.tile([128, GROUP_T], x_dt, tag="tp")
                for t in range(TPG):
                    xv = tiles[g][t][:].rearrange("p (a b) -> p b a", b=N_CHUNKS)
                    nc.tensor.transpose(
                        pt[:, t * 128:(t + 1) * 128],
                        xv[:, c, :],
                        ident_r[:],
                    )
                hi = xtpool.tile([128, GROUP_T], _F32R, tag="xhi")
                if MODE != "split3":
                    nc.vector.tensor_copy(hi[:, 0:H], pt[:, 0:H])
                    nc.scalar.copy(out=hi[:, H:], in_=pt[:, H:])
                    return (hi,)
                nc.scalar.copy(out=hi[:], in_=pt[:])
                lo = xtpool.tile([128, GROUP_T], _F32R, tag="xlo")
                nc.vector.tensor_tensor(
                    out=lo[:], in0=pt[:], in1=hi[:].bitcast(_F32),
                    op=mybir.AluOpType.subtract,
                )
                return (hi, lo)

            n_mm = 3 if MODE == "split3" else 1
            NTOT = N_GROUPS * N_CHUNKS
            slabs = {}
            for f in range(SKEW):
                slabs[f] = make_xt(f // N_CHUNKS, f % N_CHUNKS)

            for g in range(N_GROUPS):
                logits_ps = ps_mm.tile([E, GROUP_T], _F32, tag="lg")
                for c in range(N_CHUNKS):
                    flat = g * N_CHUNKS + c
                    ahead = flat + SKEW
                    if ahead < NTOT:
                        g2, c2 = divmod(ahead, N_CHUNKS)
                        if c2 == 0:
                            load_group(g2 + 1)
                        slabs[ahead] = make_xt(g2, c2)
                    sl = slabs.pop(flat)
                    if MODE == "split3":
                        hi, lo = sl
                        parts = [
                            (wt_hi[:, c, :], hi[:]),
                            (wt_hi[:, c, :], lo[:]),
                            (wt_lo[:, c, :], hi[:]),
                        ]
                    else:
                        parts = [(wt_hi[:, c, :], sl[0][:])]
                    for j, (lhs, rhs) in enumerate(parts):
                        mm_i = c * n_mm + j
                        nc.tensor.matmul(
                            logits_ps[:], lhs, rhs,
                            start=(mm_i == 0),
                            stop=(mm_i == N_CHUNKS * n_mm - 1),
                        )

                # epilogue: transpose logitsT back to [tokens, E] in one PSUM
                # tile, then per-tile top-8 and one group-batched softmax
                lg_sb = spool.tile([E, GROUP_T], _F32, tag="lgsb")
                nc.scalar.copy(out=lg_sb[:], in_=logits_ps[:])
                lt_ps = ps_lt.tile([128, TPG, E], _F32, tag="lt")
                for t in range(TPG):
                    nc.tensor.transpose(
                        lt_ps[:, t, :],
                        lg_sb[:, t * 128:(t + 1) * 128],
                        ident[0:E, 0:E],
                    )
                lg_t = spool.tile([128, TPG, E], _F32, tag="lgt")
                nc.vector.tensor_copy(lg_t[:], lt_ps[:])

                m8 = tpool.tile([128, TPG, TOPK], _F32, tag="m8")
                i8 = tpool.tile([128, TPG, TOPK], _U32, tag="i8")
                for t in range(TPG):
                    nc.vector.max(out=m8[:, t, :], in_=lg_t[:, t, :])
                    nc.vector.max_index(
                        out=i8[:, t, :], in_max=m8[:, t, :], in_values=lg_t[:, t, :]
                    )

                # softmax over the top-8, batched across the 4 token tiles
                e8 = tpool.tile([128, TPG, TOPK], _F32, tag="e8")
                nc.scalar.activation(
                    e8[:], m8[:],
                    mybir.ActivationFunctionType.Exp, scale=1.0,
                )
                s1 = tpool.tile([128, TPG, 1], _F32, tag="s1")
                nc.vector.reduce_sum(s1[:], e8[:], axis=mybir.AxisListType.X)
                rc = tpool.tile([128, TPG, 1], _F32, tag="rc")
                nc.vector.reciprocal(rc[:], s1[:])
                w8 = tpool.tile([128, TPG, TOPK], _F32, tag="w8")
                nc.vector.tensor_tensor(
                    out=w8[:], in0=e8[:],
                    in1=rc[:].broadcast_to([128, TPG, TOPK]),
                    op=mybir.AluOpType.mult,
                )

                nc.sync.dma_start(
                    out=topw.rearrange("(n p) k -> p n k", p=128)[:, g * TPG:(g + 1) * TPG, :],
                    in_=w8[:],
                )
                nc.sync.dma_start(
                    out=topi.rearrange("(n p) k -> p n k", p=128)[:, g * TPG:(g + 1) * TPG, :],
                    in_=i8[:],
                )

    nc.compile()
    return nc


_NC_CACHE = {}


def _get_nc():
    if "nc" not in _NC_CACHE:
        _NC_CACHE["nc"] = _build()
    return _NC_CACHE["nc"]


def kernel(x: np.ndarray, weight: np.ndarray, _trace=False, _trace_kwargs=None):
    assert x.shape == (4, 4096, D) and weight.shape == (E, D)
    xf = np.ascontiguousarray(x.reshape(T_FULL, D), dtype=np.float32)
    wTv = np.ascontiguousarray(weight.astype(np.float32, copy=False).T)

    nc = _get_nc()
    in_maps = [
        {"x": xf[k * T_LOC:(k + 1) * T_LOC], "wT": wTv}
        for k in range(N_CORES)
    ]
    res = run_bass_kernel_spmd(
        nc, in_maps, list(range(N_CORES)),
        trace=_trace, **(_trace_kwargs or {}),
    )
    topw = np.concatenate([res.results[k]["topw"] for k in range(N_CORES)], axis=0)
    topi = np.concatenate(
        [res.results[k]["topi"].astype(np.int32) for k in range(N_CORES)], axis=0
    )
    if _trace:
        kernel.last_exec_time_ns = res.exec_time_ns
        kernel.last_results = res
    return topw, topi
